# revision 3
# baseline (speedup 1.0000x reference)
"""BERT attention block (quirky variant: energies=Q@V^T, attended=W@K) on 8 trn2
NeuronCores — v3.

Sharding: zero-collective. Core c handles batch b=c//4, query rows
[512*(c%4), 512*(c%4+1)). K/V projections for the batch are duplicated across
the 4 cores of a batch (cheaper than collectives here).

v3 vs v2:
  - Single static PSUM pool (tags pe/pa/pv) for the whole kernel; K-proj,
    Q-proj, V-proj, attention, and the output projection all draw from the
    same tag rings -> zero mid-kernel PSUM pool transitions (each cost 2-7us
    in drains/barriers).
  - K-proj first (its inputs land earliest on the scalar/gpsimd queues),
    then Q, then V pairs 0-1; V pairs 2-7 ride the attention as PE filler.
  - One SBUF transition: {wq, embq, wk} pool released after the projections,
    Wo/res prefetched into the gap. embT/Wv stay resident to the end.
  - V(7) split across pairs 5 and 6 so the ACT-bound late pairs keep the PE
    fed longer.
"""

import sys

sys.path.insert(0, "/opt/trn_rl_repo")

import numpy as np

B, L, H = 2, 2048, 1024
NH, HEAD = 16, 64
NC = 8
QS = L // 4  # query rows per core
KT = H // 128  # contraction tiles for the projections
LB = L // 128  # key-position blocks
NP = NH // 2  # head pairs
LN_EPS = 1e-12

_programs = {}


def _build(has_bias, has_mask, has_gamma, has_beta, debug=False):
    import concourse.mybir as mybir
    import concourse.tile as tile
    from concourse import bacc

    F32 = mybir.dt.float32
    F32R = mybir.dt.float32r
    BF16 = mybir.dt.bfloat16
    AF = mybir.ActivationFunctionType
    AX = mybir.AxisListType
    OP = mybir.AluOpType

    nc = bacc.Bacc("TRN2", target_bir_lowering=False, debug=False, num_devices=NC)

    embq_d = nc.dram_tensor("embqS", [128, KT * QS], BF16, kind="ExternalInput")
    embt_d = nc.dram_tensor("embtS", [128, 4 * KT * 512], BF16, kind="ExternalInput")
    wq_d = nc.dram_tensor("wqS", [128, KT * H], BF16, kind="ExternalInput")
    wk_d = nc.dram_tensor("wkS", [128, KT * H], BF16, kind="ExternalInput")
    wv_d = nc.dram_tensor("wvS", [128, KT * H], BF16, kind="ExternalInput")
    wo_d = nc.dram_tensor("woS", [128, KT * H], BF16, kind="ExternalInput")
    ones_d = nc.dram_tensor("onescol", [128, 64], F32R, kind="ExternalInput")
    res_d = nc.dram_tensor("resS", [128, 4 * H], F32, kind="ExternalInput")
    if has_mask:
        mask_d = nc.dram_tensor("maskpk", [128, LB], F32, kind="ExternalInput")
    if has_bias:
        bq_d = nc.dram_tensor("bqr", [1, H], BF16, kind="ExternalInput")
        bk_d = nc.dram_tensor("bkr", [1, H], BF16, kind="ExternalInput")
        bv_d = nc.dram_tensor("bvr", [1, H], BF16, kind="ExternalInput")
        onesrow_d = nc.dram_tensor("onesrow", [1, H], BF16, kind="ExternalInput")
    if has_gamma:
        gam_d = nc.dram_tensor("gam", [128, H], F32, kind="ExternalInput")
    if has_beta:
        bet_d = nc.dram_tensor("bet", [128, H], F32, kind="ExternalInput")
    out_d = nc.dram_tensor("out", [QS, H], F32, kind="ExternalOutput")
    if debug:
        qt_dbg = nc.dram_tensor("qt_dbg", [128, NP * QS], BF16, kind="ExternalOutput")
        kh_dbg = nc.dram_tensor(
            "kh_dbg", [128, LB * NH * 65], BF16, kind="ExternalOutput"
        )
        vt_dbg = nc.dram_tensor("vt_dbg", [128, NP * L], BF16, kind="ExternalOutput")
        apr_dbg = nc.dram_tensor(
            "apr_dbg", [128, NP * QS], BF16, kind="ExternalOutput"
        )

    with tile.TileContext(nc) as tc:
        with tc.tile_pool(name="persist", bufs=1) as pp:
            ones16 = pp.tile([128, 64], F32R, name="ones16")
            if has_mask:
                maskt = pp.tile([128, LB], F32, name="maskt")
            if has_bias:
                ones1 = pp.tile([1, H], BF16, name="ones1")
                nc.gpsimd.dma_start(ones1[:], onesrow_d[:])
                bqr = pp.tile([1, H], BF16, name="bqr")
                bkr = pp.tile([1, H], BF16, name="bkr")
                bvr = pp.tile([1, H], BF16, name="bvr")
                nc.gpsimd.dma_start(bqr[:], bq_d[:])
                nc.gpsimd.dma_start(bkr[:], bk_d[:])
                nc.gpsimd.dma_start(bvr[:], bv_d[:])
            # long-lived activation tensors (bf16)
            qtall = pp.tile([128, NP * QS], BF16, name="qtall")
            khall = pp.tile([128, LB * NH * 65], BF16, name="khall")
            vtall = pp.tile([128, NP * L], BF16, name="vtall")
            aprall = pp.tile([128, NP * QS], BF16, name="aprall")

            qt = lambda p: qtall[:, QS * p : QS * (p + 1)]  # noqa: E731
            kh = lambda lb: khall[:, NH * 65 * lb : NH * 65 * (lb + 1)]  # noqa: E731
            vt = lambda p: vtall[:, L * p : L * (p + 1)]  # noqa: E731
            apr = lambda p: aprall[:, QS * p : QS * (p + 1)]  # noqa: E731

            # embT + Wv: resident to the end (attention fillers read them)
            jp = tc.alloc_tile_pool(name="proj", bufs=1, side="right")
            embtall = jp.tile([128, KT * L], BF16, name="embtall")
            wvall = jp.tile([128, KT * H], BF16, name="wvall")
            embt = lambda i: embtall[:, L * i : L * (i + 1)]  # noqa: E731
            wvt = lambda i: wvall[:, H * i : H * (i + 1)]  # noqa: E731

            # fin pool up-front: wkall occupies the slot that Wo takes over
            # later (same tag ring -> WAR semaphores, no pool-release drains)
            fin = tc.alloc_tile_pool(name="fin", bufs=1, side="right")
            wkall = fin.tile([128, KT * H], BF16, tag="ovl", bufs=1, name="wkall")
            resall = fin.tile([128, 4 * H], F32, name="resall")
            epst = fin.tile([128, 1], F32, name="epst")

            # scoped Q-proj inputs: released before attention
            qkw = tc.alloc_tile_pool(name="qkw", bufs=1)
            wqall = qkw.tile([128, KT * H], BF16, name="wqall")
            embqall = qkw.tile([128, KT * QS], BF16, name="embqall")

            # DMA issue order: stripe each early tensor across the three
            # queues (sync/scalar/gpsimd) so several DMA engines pull
            # concurrently (~100 GB/s per queue descriptor stream).
            def embt_q(q, eng):
                eng.dma_start(
                    embtall[:].rearrange("p (t l) -> p t l", l=L)[
                        :, :, 512 * q : 512 * (q + 1)
                    ],
                    embt_d[:, KT * 512 * q : KT * 512 * (q + 1)].rearrange(
                        "p (t l) -> p t l", l=512
                    ),
                )

            # DMA order targets the shortened prefix Q -> K(heads 0-7) ->
            # V(0,1): wq is m-major (chain g = one contiguous chunk), wk is
            # half-major (heads 0-7 = first 1MB), wv m-major (pair p = one
            # 0.25MB column group). K heads 8-15 and V(2..7) stream in during
            # attention as PE filler.
            nc.gpsimd.dma_start(embqall[:], embq_d[:])
            nc.sync.dma_start(wqall[:, : 2 * H], wq_d[:, : 2 * H])
            nc.scalar.dma_start(wqall[:, 4 * H :], wq_d[:, 4 * H :])
            nc.sync.dma_start(wqall[:, 2 * H : 4 * H], wq_d[:, 2 * H : 4 * H])
            nc.gpsimd.dma_start(wkall[:, : 2 * H], wk_d[:, : 2 * H])
            nc.sync.dma_start(wkall[:, 2 * H : 4 * H], wk_d[:, 2 * H : 4 * H])
            embt_q(0, nc.scalar)
            nc.scalar.dma_start(wvall[:, : 2 * H], wv_d[:, : 2 * H])
            embt_q(1, nc.gpsimd)
            embt_q(2, nc.sync)
            embt_q(3, nc.scalar)
            # inputs for the attention-phase fillers
            nc.gpsimd.dma_start(wkall[:, 4 * H :], wk_d[:, 4 * H :])
            nc.sync.dma_start(wvall[:, 2 * H : 5 * H], wv_d[:, 2 * H : 5 * H])
            nc.gpsimd.dma_start(wvall[:, 5 * H :], wv_d[:, 5 * H :])
            nc.gpsimd.dma_start(ones16[:], ones_d[:])
            if has_mask:
                nc.gpsimd.dma_start(maskt[:], mask_d[:])

            # single static PSUM pool: tags pe (2 banks x2), pa (2 x1),
            # pv (1 bank x2) = 8 banks, used by every phase.
            psA = tc.alloc_tile_pool(name="psA", bufs=1, space="PSUM")

            # ---- Q projection: two m-chains per pe tile (halves)
            for g in range(4):
                pq = psA.tile([128, 2 * QS], F32, tag="pe", bufs=2, name=f"pq{g}")
                for kt in range(KT):
                    for j in range(2):
                        m = 2 * g + j
                        nc.tensor.matmul(
                            pq[:, QS * j : QS * (j + 1)],
                            wqall[:, H * m + 128 * kt : H * m + 128 * (kt + 1)],
                            embqall[:, QS * kt : QS * (kt + 1)],
                            start=(kt == 0),
                            stop=(kt == KT - 1 and not has_bias),
                        )
                for j in range(2):
                    m = 2 * g + j
                    if has_bias:
                        nc.tensor.matmul(
                            pq[:, QS * j : QS * (j + 1)],
                            bqr[:, 128 * m : 128 * (m + 1)],
                            ones1[:, :QS],
                            start=False,
                            stop=True,
                        )
                    nc.scalar.copy(qt(m), pq[:, QS * j : QS * (j + 1)])

            # ---- K projection, heads 0-7 (wk is hf-major: slice
            # wkall[:, 4H*hf + 512*kt]); heads 8-15 ride attention as filler
            for lb in range(LB):
                pk = psA.tile([128, 512], F32, tag="pv", bufs=2, name=f"pk{lb}")
                for kt in range(KT):
                    nc.tensor.matmul(
                        pk[:],
                        embt(kt)[:, 128 * lb : 128 * (lb + 1)],
                        wkall[:, 512 * kt : 512 * (kt + 1)],
                        start=(kt == 0),
                        stop=(kt == KT - 1 and not has_bias),
                    )
                if has_bias:
                    nc.tensor.matmul(
                        pk[:],
                        ones1[:, 0:128],
                        bkr[:, 0:512],
                        start=False,
                        stop=True,
                    )
                dst = kh(lb).rearrange("p (h x) -> p h x", x=65)
                nc.vector.tensor_copy(
                    dst[:, 0:8, 0:64], pk[:].rearrange("p (h x) -> p h x", x=64)
                )
                nc.gpsimd.memset(dst[:, :, 64:65], 1.0)

            # ---- V projection, pairs 0-1 (wv is m-major: pair p's weights
            # are the contiguous column group wvall[:, H*p + 128*kt])
            def vproj_big(p):
                for hf in range(2):
                    pv = psA.tile(
                        [128, 2 * QS], F32, tag="pe", bufs=2, name=f"pv{p}_{hf}"
                    )
                    for kt in range(KT):
                        for qh in range(2):
                            nc.tensor.matmul(
                                pv[:, 512 * qh : 512 * (qh + 1)],
                                wvall[:, H * p + 128 * kt : H * p + 128 * (kt + 1)],
                                embt(kt)[
                                    :,
                                    H * hf + 512 * qh : H * hf + 512 * (qh + 1),
                                ],
                                start=(kt == 0),
                                stop=(kt == KT - 1 and not has_bias),
                            )
                    if has_bias:
                        for qh in range(2):
                            nc.tensor.matmul(
                                pv[:, 512 * qh : 512 * (qh + 1)],
                                bvr[:, 128 * p : 128 * (p + 1)],
                                ones1[:, :512],
                                start=False,
                                stop=True,
                            )
                    nc.scalar.copy(vt(p)[:, H * hf : H * (hf + 1)], pv[:])

            vproj_big(0)
            vproj_big(1)

            # Q inputs die; Wo takes over wk's slot mid-attention (WAR sem)
            qkw.release()
            woall = fin.tile([128, KT * H], BF16, tag="ovl", bufs=1, name="woall")
            nc.scalar.dma_start(woall[:], wo_d[:])
            nc.scalar.dma_start(resall[:], res_d[:])
            if has_gamma:
                gam = fin.tile([128, H], F32, name="gam")
                nc.scalar.dma_start(gam[:], gam_d[:])
            if has_beta:
                bet = fin.tile([128, H], F32, name="bet")
                nc.scalar.dma_start(bet[:], bet_d[:])
            nc.gpsimd.memset(epst[:], LN_EPS)

            ap = tc.alloc_tile_pool(name="attn", bufs=1)

            pv_box = [None]

            def v_quantum(tp, c, j):
                """Two V-proj matmuls for target pair tp, seq chunk c."""
                if j == 0:
                    pv_box[0] = psA.tile(
                        [128, 512], F32, tag="pv", bufs=2, name=f"pvf{tp}_{c}"
                    )
                pv = pv_box[0]
                for kt in (2 * j, 2 * j + 1):
                    nc.tensor.matmul(
                        pv[:],
                        wvall[:, H * tp + 128 * kt : H * tp + 128 * (kt + 1)],
                        embt(kt)[:, 512 * c : 512 * (c + 1)],
                        start=(kt == 0),
                        stop=(kt == KT - 1 and not has_bias),
                    )
                if j == 3:
                    if has_bias:
                        nc.tensor.matmul(
                            pv[:],
                            bvr[:, 128 * tp : 128 * (tp + 1)],
                            ones1[:, :512],
                            start=False,
                            stop=True,
                        )
                    nc.vector.tensor_copy(
                        vt(tp)[:, 512 * c : 512 * (c + 1)], pv[:]
                    )

            def k_quantum(lb, j):
                """Two K-proj (heads 8-15) matmuls for key block lb."""
                if j == 0:
                    pv_box[0] = psA.tile(
                        [128, 512], F32, tag="pv", bufs=2, name=f"pkf{lb}"
                    )
                pk = pv_box[0]
                for kt in (2 * j, 2 * j + 1):
                    nc.tensor.matmul(
                        pk[:],
                        embt(kt)[:, 128 * lb : 128 * (lb + 1)],
                        wkall[:, 4 * H + 512 * kt : 4 * H + 512 * (kt + 1)],
                        start=(kt == 0),
                        stop=(kt == KT - 1 and not has_bias),
                    )
                if j == 3:
                    if has_bias:
                        nc.tensor.matmul(
                            pk[:],
                            ones1[:, 0:128],
                            bkr[:, 512:1024],
                            start=False,
                            stop=True,
                        )
                    nc.vector.tensor_copy(
                        kh(lb).rearrange("p (h x) -> p h x", x=65)[:, 8:16, 0:64],
                        pk[:].rearrange("p (h x) -> p h x", x=64),
                    )

            def v_chain(tp, cs):
                return [
                    (lambda tp=tp, c=c, j=j: v_quantum(tp, c, j))
                    for c in cs
                    for j in range(4)
                ]

            def k_chain(lbs):
                return [
                    (lambda lb=lb, j=j: k_quantum(lb, j))
                    for lb in lbs
                    for j in range(4)
                ]

            # pair 7 runs mid-sequence so its normalization (and every
            # out-proj t=7 matmul) resolves during attention; only pair 6's
            # norm lands in the tail. Filler work-queues keyed by POSITION;
            # each target pair's V (and kh heads 8-15 for pair 7 at pos 2)
            # is complete before that pair runs.
            pair_order = [0, 1, 7, 2, 3, 4, 5, 6]
            fill_by_pos = {
                0: k_chain(range(0, 8)) + v_chain(7, [0, 1]),
                1: k_chain(range(8, 16)) + v_chain(7, [2, 3]),
                2: v_chain(2, [0, 1, 2, 3]),
                3: v_chain(3, [0, 1, 2, 3]),
                4: v_chain(4, [0, 1, 2, 3]),
                5: v_chain(5, [0, 1, 2, 3]),
                6: v_chain(6, [0, 1, 2, 3]),
            }

            def emit_E(p, kb):
                pe2 = psA.tile(
                    [128, 2 * QS], F32, tag="pe", bufs=2, name=f"pe{p}_{kb}"
                )
                nc.tensor.matmul(
                    pe2[:, 0:QS],
                    vt(p)[0:64, 128 * kb : 128 * (kb + 1)],
                    qt(p)[0:64, :],
                    start=True,
                    stop=True,
                )
                nc.tensor.matmul(
                    pe2[:, QS : 2 * QS],
                    vt(p)[64:128, 128 * kb : 128 * (kb + 1)],
                    qt(p)[64:128, :],
                    start=True,
                    stop=True,
                )
                ex = ap.tile([128, 2 * QS], BF16, tag="ex", bufs=3, name=f"ex{p}_{kb}")
                bias = maskt[:, kb : kb + 1] if has_mask else 0.0
                nc.scalar.activation(ex[:], pe2[:], AF.Exp, bias=bias, scale=0.25)
                return ex

            def emit_att(p, kb, ex, pa):
                h0, h1 = 2 * p, 2 * p + 1
                nc.tensor.matmul(
                    pa[:, 0:QS],
                    kh(kb)[:, 65 * h0 : 65 * (h0 + 1)],
                    ex[:, 0:QS],
                    start=(kb == 0),
                    stop=(kb == LB - 1),
                )
                nc.tensor.matmul(
                    pa[:, QS : 2 * QS],
                    kh(kb)[:, 65 * h1 : 65 * (h1 + 1)],
                    ex[:, QS : 2 * QS],
                    start=(kb == 0),
                    stop=(kb == LB - 1),
                )

            def norm_stageBC(p, rcr):
                """PE broadcast of 1/sumexp + in-place normalize of apr(p).
                Deferred into the NEXT pair's kb loop so the PE never waits
                on the DVE reciprocal chain."""
                pb = psA.tile([128, 2 * QS], F32, tag="pe", bufs=2, name=f"pb{p}")
                nc.tensor.matmul(
                    pb[0:64, 0:QS],
                    ones16[64:65, 0:64],
                    rcr[64:65, 0:QS],
                    start=True,
                    stop=True,
                )
                nc.tensor.matmul(
                    pb[0:64, QS : 2 * QS],
                    ones16[64:65, 0:64],
                    rcr[64:65, QS : 2 * QS],
                    start=True,
                    stop=True,
                )
                nc.vector.tensor_mul(
                    apr(p)[0:64, :], apr(p)[0:64, :], pb[0:64, 0:QS]
                )
                nc.vector.tensor_mul(
                    apr(p)[64:128, :], apr(p)[64:128, :], pb[0:64, QS : 2 * QS]
                )

            pending_norm = [None]
            for pos in range(NP):
                p = pair_order[pos]
                pa = psA.tile([65, 2 * QS], F32, tag="pa", bufs=1, name=f"pa{p}")
                exs = [None] * LB
                work = fill_by_pos.get(pos, [])
                for kb in range(LB):
                    exs[kb] = emit_E(p, kb)
                    if kb == LB - 1 and pending_norm[0] is not None:
                        # fire the previous pair's norm here: its pe-ring
                        # slot follows pe2(kb15), so it waits only exp(kb14)
                        # and its rcr input is long ready -> no PE stall.
                        pending_norm[0]()
                        pending_norm[0] = None
                    if kb > 0:
                        emit_att(p, kb - 1, exs[kb - 1], pa)
                    for qi in range(
                        kb * len(work) // LB, (kb + 1) * len(work) // LB
                    ):
                        work[qi]()
                emit_att(p, LB - 1, exs[LB - 1], pa)
                # stage A (DVE + DMA only): evacuate unnormalized attended,
                # reciprocal of the ones-row sums, round to f32r.
                sc = ap.tile([65, 2 * QS], BF16, tag="sc", bufs=2, name=f"sc{p}")
                nc.vector.tensor_copy(sc[0:64, :], pa[0:64, :])
                rcw = ap.tile([65, 4 * QS], F32, tag="rcw", bufs=1, name=f"rcw{p}")
                nc.vector.tensor_copy(rcw[:, 2 * QS :], pa[:, :])
                nc.vector.reciprocal_approx_fast(
                    rcw[:, 0 : 2 * QS], rcw[:, 2 * QS :]
                )
                rcr = ap.tile([65, 2 * QS], F32R, tag="rcr", bufs=1, name=f"rcr{p}")
                nc.vector.tensor_copy(rcr[64:65, :], rcw[64:65, 0 : 2 * QS])
                nc.sync.dma_start(apr(p)[0:64, :], sc[0:64, 0:QS])
                nc.sync.dma_start(apr(p)[64:128, :], sc[0:64, QS : 2 * QS])
                pending_norm[0] = lambda p=p, rcr=rcr: norm_stageBC(p, rcr)

            if debug:
                nc.sync.dma_start(qt_dbg[:], qtall[:])
                nc.sync.dma_start(kh_dbg[:], khall[:])
                nc.sync.dma_start(vt_dbg[:], vtall[:])

            # ---- output projection + residual + LayerNorm
            wot = lambda t: woall[:, H * t : H * (t + 1)]  # noqa: E731

            t_order = [0, 1, 2, 3, 4, 5, 7, 6]
            for qcb in range(4):
                pos = [
                    psA.tile(
                        [128, 512], F32, tag="pv", bufs=2, name=f"po{qcb}_{hf}"
                    )
                    for hf in range(2)
                ]
                for ti, t in enumerate(t_order):
                    if qcb == 0 and ti == NP - 1:
                        # pair-6 (last attention pair) normalization, nested
                        # here so earlier po matmuls hoist into the ACT-bound
                        # late pairs
                        pending_norm[0]()
                        pending_norm[0] = None
                    for hf in range(2):
                        nc.tensor.matmul(
                            pos[hf][:],
                            apr(t)[:, 128 * qcb : 128 * (qcb + 1)],
                            wot(t)[:, 512 * hf : 512 * (hf + 1)],
                            start=(ti == 0),
                            stop=(ti == NP - 1),
                        )
                # LayerNorm: sums fused into the residual add; Square and the
                # final scale take the (negated) mean as an ACT bias AP.
                x = fin.tile([128, H], F32, tag="x", bufs=1, name=f"x{qcb}")
                xs = fin.tile([128, 2], F32, tag="xs", bufs=2, name=f"xs{qcb}")
                for hf in range(2):
                    nc.vector.scalar_tensor_tensor(
                        x[:, 512 * hf : 512 * (hf + 1)],
                        pos[hf][:],
                        1.0,
                        resall[:, H * qcb + 512 * hf : H * qcb + 512 * (hf + 1)],
                        op0=OP.mult,
                        op1=OP.add,
                        accum_out=xs[:, hf : hf + 1],
                    )
                mun = fin.tile([128, 1], F32, tag="mun", bufs=2, name=f"mun{qcb}")
                nc.vector.tensor_add(mun[:], xs[:, 0:1], xs[:, 1:2])
                nc.vector.tensor_scalar_mul(mun[:], mun[:], -1.0 / H)
                sq = fin.tile([128, H], F32, tag="sq", bufs=1, name=f"sq{qcb}")
                var = fin.tile([128, 1], F32, tag="var", bufs=2, name=f"var{qcb}")
                nc.scalar.activation(
                    sq[:], x[:], AF.Square, bias=mun[:], accum_out=var[:]
                )
                std = fin.tile([128, 1], F32, tag="std", bufs=2, name=f"std{qcb}")
                nc.scalar.activation(
                    std[:], var[:], AF.Sqrt, scale=1.0 / H, bias=epst[:]
                )
                rstd = fin.tile([128, 1], F32, tag="rstd", bufs=2, name=f"rstd{qcb}")
                nc.vector.reciprocal(rstd[:], std[:])
                b2 = fin.tile([128, 1], F32, tag="b2", bufs=2, name=f"b2{qcb}")
                nc.vector.tensor_mul(b2[:], mun[:], rstd[:])
                y = fin.tile([128, H], F32, tag="y", bufs=2, name=f"y{qcb}")
                nc.scalar.activation(
                    y[:], x[:], AF.Identity, bias=b2[:], scale=rstd[:]
                )
                if has_gamma:
                    nc.vector.tensor_mul(y[:], y[:], gam[:])
                if has_beta:
                    nc.vector.tensor_add(y[:], y[:], bet[:])
                oeng = nc.sync if qcb % 2 == 0 else nc.gpsimd
                oeng.dma_start(out_d[128 * qcb : 128 * (qcb + 1), :], y[:])
            if debug:
                nc.sync.dma_start(apr_dbg[:], aprall[:])
            psA.release()
            ap.release()
            fin.release()
            jp.release()

    nc.compile()
    return nc


def _bf16():
    import concourse.mybir as mybir

    return mybir.dt.np(mybir.dt.bfloat16)


def _tile_major(a, nt):
    """[nt*128, F] row-major -> [128, nt*F] tile-major (bf16)."""
    f = a.shape[1]
    return np.ascontiguousarray(
        a.reshape(nt, 128, f).transpose(1, 0, 2).reshape(128, nt * f)
    )


def stage_inputs(embeddings, mask, Wq, bq, Wk, bk, Wv, bv, Wo, bo, ln_gamma, ln_beta):
    """Build per-core in_maps (host-side layout staging)."""
    bf16 = _bf16()
    embeddings = np.asarray(embeddings, dtype=np.float32)
    mask = np.asarray(mask, dtype=np.float32)
    Wq, bq = np.asarray(Wq, np.float32), np.asarray(bq, np.float32)
    Wk, bk = np.asarray(Wk, np.float32), np.asarray(bk, np.float32)
    Wv, bv = np.asarray(Wv, np.float32), np.asarray(bv, np.float32)
    Wo, bo = np.asarray(Wo, np.float32), np.asarray(bo, np.float32)
    ln_gamma = np.asarray(ln_gamma, np.float32)
    ln_beta = np.asarray(ln_beta, np.float32)

    has_bias = bool(np.any(bq) or np.any(bk) or np.any(bv))
    has_mask = bool(np.any(mask))
    has_gamma = bool(np.any(ln_gamma != 1.0))
    has_beta = bool(np.any(ln_beta))
    key = (has_bias, has_mask, has_gamma, has_beta)

    # wq staged m-major: [p, m*H + kt*128 + c] = Wq[kt*128+p, m*128+c]
    wq_s = np.ascontiguousarray(
        Wq.astype(bf16)
        .reshape(KT, 128, KT, 128)
        .transpose(1, 2, 0, 3)
        .reshape(128, KT * H)
    )
    # wk staged hf-major: [p, hf*4H + kt*512 + c] = Wk[kt*128+p, hf*512+c]
    wk_s = np.ascontiguousarray(
        Wk.astype(bf16)
        .reshape(KT, 128, 2, 512)
        .transpose(1, 2, 0, 3)
        .reshape(128, KT * H)
    )
    # wv staged m-major like wq
    wv_s = np.ascontiguousarray(
        Wv.astype(bf16)
        .reshape(KT, 128, KT, 128)
        .transpose(1, 2, 0, 3)
        .reshape(128, KT * H)
    )
    wo_s = _tile_major(Wo.astype(bf16), KT)
    ones_s = np.ones((128, 64), dtype=np.float32)

    in_maps = []
    for c in range(NC):
        b, s = c // 4, c % 4
        e = embeddings[b]  # (L, H)
        embT = np.ascontiguousarray(e.T).astype(bf16)  # (H, L) bf16
        embq_s = _tile_major(
            np.ascontiguousarray(embT[:, QS * s : QS * (s + 1)]), KT
        )
        # embt: quarter-major [128, (q t l)] so each quarter is 1 contiguous DMA
        embt_s = np.ascontiguousarray(
            embT.reshape(KT, 128, 4, 512)
            .transpose(1, 2, 0, 3)
            .reshape(128, 4 * KT * 512)
        )
        res = (e[QS * s : QS * (s + 1)] + bo[None, :]).astype(np.float32)
        res_s = np.ascontiguousarray(
            res.reshape(4, 128, H).transpose(1, 0, 2).reshape(128, 4 * H)
        )
        m = {
            "embqS": embq_s,
            "embtS": embt_s,
            "wqS": wq_s,
            "wkS": wk_s,
            "wvS": wv_s,
            "woS": wo_s,
            "onescol": ones_s,
            "resS": res_s,
        }
        if has_mask:
            m["maskpk"] = np.ascontiguousarray(mask[b, 0, 0].reshape(LB, 128).T)
        if has_bias:
            m["bqr"] = bq[None, :].astype(bf16)
            m["bkr"] = bk[None, :].astype(bf16)
            m["bvr"] = bv[None, :].astype(bf16)
            m["onesrow"] = np.ones((1, H), dtype=bf16)
        if has_gamma:
            m["gam"] = np.broadcast_to(ln_gamma, (128, H)).astype(np.float32).copy()
        if has_beta:
            m["bet"] = np.broadcast_to(ln_beta, (128, H)).astype(np.float32).copy()
        in_maps.append(m)
    return key, in_maps


def kernel(embeddings, mask, Wq, bq, Wk, bk, Wv, bv, Wo, bo, ln_gamma, ln_beta):
    from concourse.bass_utils import run_bass_kernel_spmd

    key, in_maps = stage_inputs(
        embeddings, mask, Wq, bq, Wk, bk, Wv, bv, Wo, bo, ln_gamma, ln_beta
    )
    if key not in _programs:
        _programs[key] = _build(*key)
    nc = _programs[key]

    r = run_bass_kernel_spmd(nc, in_maps, list(range(NC)))
    out = np.empty((B, L, H), dtype=np.float32)
    for c in range(NC):
        b, s = c // 4, c % 4
        out[b, QS * s : QS * (s + 1)] = r.results[c]["out"]
    return out


# revision 4
# speedup vs baseline: 1.0128x; 1.0128x over previous
"""BERT attention block (quirky variant: energies=Q@V^T, attended=W@K) on 8 trn2
NeuronCores — v3.

Sharding: zero-collective. Core c handles batch b=c//4, query rows
[512*(c%4), 512*(c%4+1)). K/V projections for the batch are duplicated across
the 4 cores of a batch (cheaper than collectives here).

v3 vs v2:
  - Single static PSUM pool (tags pe/pa/pv) for the whole kernel; K-proj,
    Q-proj, V-proj, attention, and the output projection all draw from the
    same tag rings -> zero mid-kernel PSUM pool transitions (each cost 2-7us
    in drains/barriers).
  - K-proj first (its inputs land earliest on the scalar/gpsimd queues),
    then Q, then V pairs 0-1; V pairs 2-7 ride the attention as PE filler.
  - One SBUF transition: {wq, embq, wk} pool released after the projections,
    Wo/res prefetched into the gap. embT/Wv stay resident to the end.
  - V(7) split across pairs 5 and 6 so the ACT-bound late pairs keep the PE
    fed longer.
"""

import sys

sys.path.insert(0, "/opt/trn_rl_repo")

import numpy as np

B, L, H = 2, 2048, 1024
NH, HEAD = 16, 64
NC = 8
QS = L // 4  # query rows per core
KT = H // 128  # contraction tiles for the projections
LB = L // 128  # key-position blocks
NP = NH // 2  # head pairs
LN_EPS = 1e-12

_programs = {}


def _build(has_bias, has_mask, has_gamma, has_beta, debug=False):
    import concourse.mybir as mybir
    import concourse.tile as tile
    from concourse import bacc

    F32 = mybir.dt.float32
    F32R = mybir.dt.float32r
    BF16 = mybir.dt.bfloat16
    AF = mybir.ActivationFunctionType
    AX = mybir.AxisListType
    OP = mybir.AluOpType

    nc = bacc.Bacc("TRN2", target_bir_lowering=False, debug=False, num_devices=NC)

    embq_d = nc.dram_tensor("embqS", [128, KT * QS], BF16, kind="ExternalInput")
    embt_d = nc.dram_tensor("embtS", [128, 4 * KT * 512], BF16, kind="ExternalInput")
    wq_d = nc.dram_tensor("wqS", [128, KT * H], BF16, kind="ExternalInput")
    wk_d = nc.dram_tensor("wkS", [128, KT * H], BF16, kind="ExternalInput")
    wv_d = nc.dram_tensor("wvS", [128, KT * H], BF16, kind="ExternalInput")
    wo_d = nc.dram_tensor("woS", [128, KT * H], BF16, kind="ExternalInput")
    ones_d = nc.dram_tensor("onescol", [128, 64], F32R, kind="ExternalInput")
    res_d = nc.dram_tensor("resS", [128, 4 * H], F32, kind="ExternalInput")
    if has_mask:
        mask_d = nc.dram_tensor("maskpk", [128, LB], F32, kind="ExternalInput")
    if has_bias:
        bq_d = nc.dram_tensor("bqr", [1, H], BF16, kind="ExternalInput")
        bk_d = nc.dram_tensor("bkr", [1, H], BF16, kind="ExternalInput")
        bv_d = nc.dram_tensor("bvr", [1, H], BF16, kind="ExternalInput")
        onesrow_d = nc.dram_tensor("onesrow", [1, H], BF16, kind="ExternalInput")
    if has_gamma:
        gam_d = nc.dram_tensor("gam", [128, H], F32, kind="ExternalInput")
    if has_beta:
        bet_d = nc.dram_tensor("bet", [128, H], F32, kind="ExternalInput")
    out_d = nc.dram_tensor("out", [QS, H], F32, kind="ExternalOutput")
    if debug:
        qt_dbg = nc.dram_tensor("qt_dbg", [128, NP * QS], BF16, kind="ExternalOutput")
        kh_dbg = nc.dram_tensor(
            "kh_dbg", [128, LB * NH * 65], BF16, kind="ExternalOutput"
        )
        vt_dbg = nc.dram_tensor("vt_dbg", [128, NP * L], BF16, kind="ExternalOutput")
        apr_dbg = nc.dram_tensor(
            "apr_dbg", [128, NP * QS], BF16, kind="ExternalOutput"
        )

    with tile.TileContext(nc) as tc:
        with tc.tile_pool(name="persist", bufs=1) as pp:
            ones16 = pp.tile([128, 64], F32R, name="ones16")
            if has_mask:
                maskt = pp.tile([128, LB], F32, name="maskt")
            if has_bias:
                ones1 = pp.tile([1, H], BF16, name="ones1")
                nc.gpsimd.dma_start(ones1[:], onesrow_d[:])
                bqr = pp.tile([1, H], BF16, name="bqr")
                bkr = pp.tile([1, H], BF16, name="bkr")
                bvr = pp.tile([1, H], BF16, name="bvr")
                nc.gpsimd.dma_start(bqr[:], bq_d[:])
                nc.gpsimd.dma_start(bkr[:], bk_d[:])
                nc.gpsimd.dma_start(bvr[:], bv_d[:])
            # long-lived activation tensors (bf16)
            qtall = pp.tile([128, NP * QS], BF16, name="qtall")
            khall = pp.tile([128, LB * NH * 65], BF16, name="khall")
            vtall = pp.tile([128, NP * L], BF16, name="vtall")
            aprall = pp.tile([128, NP * QS], BF16, name="aprall")

            qt = lambda p: qtall[:, QS * p : QS * (p + 1)]  # noqa: E731
            kh = lambda lb: khall[:, NH * 65 * lb : NH * 65 * (lb + 1)]  # noqa: E731
            vt = lambda p: vtall[:, L * p : L * (p + 1)]  # noqa: E731
            apr = lambda p: aprall[:, QS * p : QS * (p + 1)]  # noqa: E731

            # embT + Wv: resident to the end (attention fillers read them)
            jp = tc.alloc_tile_pool(name="proj", bufs=1, side="right")
            embtall = jp.tile([128, KT * L], BF16, name="embtall")
            wvall = jp.tile([128, KT * H], BF16, name="wvall")
            embt = lambda i: embtall[:, L * i : L * (i + 1)]  # noqa: E731
            wvt = lambda i: wvall[:, H * i : H * (i + 1)]  # noqa: E731

            # fin pool up-front: wkall occupies the slot that Wo takes over
            # later (same tag ring -> WAR semaphores, no pool-release drains)
            fin = tc.alloc_tile_pool(name="fin", bufs=1, side="right")
            wkall = fin.tile([128, KT * H], BF16, tag="ovl", bufs=1, name="wkall")
            resall = fin.tile([128, 4 * H], F32, name="resall")
            epst = fin.tile([128, 1], F32, name="epst")

            # scoped Q-proj inputs: released before attention
            qkw = tc.alloc_tile_pool(name="qkw", bufs=1)
            wqall = qkw.tile([128, KT * H], BF16, name="wqall")
            embqall = qkw.tile([128, KT * QS], BF16, name="embqall")

            # DMA issue order: stripe each early tensor across the three
            # queues (sync/scalar/gpsimd) so several DMA engines pull
            # concurrently (~100 GB/s per queue descriptor stream).
            def embt_q(q, eng):
                eng.dma_start(
                    embtall[:].rearrange("p (t l) -> p t l", l=L)[
                        :, :, 512 * q : 512 * (q + 1)
                    ],
                    embt_d[:, KT * 512 * q : KT * 512 * (q + 1)].rearrange(
                        "p (t l) -> p t l", l=512
                    ),
                )

            # DMA order targets the shortened prefix Q -> K(heads 0-7) ->
            # V(0,1): wq is m-major (chain g = one contiguous chunk), wk is
            # half-major (heads 0-7 = first 1MB), wv m-major (pair p = one
            # 0.25MB column group). K heads 8-15 and V(2..7) stream in during
            # attention as PE filler.
            nc.gpsimd.dma_start(embqall[:], embq_d[:])
            nc.sync.dma_start(wqall[:, : 2 * H], wq_d[:, : 2 * H])
            nc.scalar.dma_start(wqall[:, 4 * H :], wq_d[:, 4 * H :])
            nc.sync.dma_start(wqall[:, 2 * H : 4 * H], wq_d[:, 2 * H : 4 * H])
            nc.gpsimd.dma_start(wkall[:, : 2 * H], wk_d[:, : 2 * H])
            nc.sync.dma_start(wkall[:, 2 * H : 4 * H], wk_d[:, 2 * H : 4 * H])
            embt_q(0, nc.scalar)
            nc.scalar.dma_start(wvall[:, : 2 * H], wv_d[:, : 2 * H])
            embt_q(1, nc.gpsimd)
            embt_q(2, nc.sync)
            embt_q(3, nc.scalar)
            # inputs for the attention-phase fillers
            nc.gpsimd.dma_start(wkall[:, 4 * H :], wk_d[:, 4 * H :])
            nc.sync.dma_start(wvall[:, 2 * H : 5 * H], wv_d[:, 2 * H : 5 * H])
            nc.gpsimd.dma_start(wvall[:, 5 * H :], wv_d[:, 5 * H :])
            nc.gpsimd.dma_start(ones16[:], ones_d[:])
            if has_mask:
                nc.gpsimd.dma_start(maskt[:], mask_d[:])

            # single static PSUM pool: tags pe (2 banks x2), pa (2 x1),
            # pv (1 bank x2) = 8 banks, used by every phase.
            psA = tc.alloc_tile_pool(name="psA", bufs=1, space="PSUM")

            # ---- Q projection: two m-chains per pe tile (halves)
            for g in range(4):
                pq = psA.tile([128, 2 * QS], F32, tag="pe", bufs=2, name=f"pq{g}")
                for kt in range(KT):
                    for j in range(2):
                        m = 2 * g + j
                        nc.tensor.matmul(
                            pq[:, QS * j : QS * (j + 1)],
                            wqall[:, H * m + 128 * kt : H * m + 128 * (kt + 1)],
                            embqall[:, QS * kt : QS * (kt + 1)],
                            start=(kt == 0),
                            stop=(kt == KT - 1 and not has_bias),
                        )
                for j in range(2):
                    m = 2 * g + j
                    if has_bias:
                        nc.tensor.matmul(
                            pq[:, QS * j : QS * (j + 1)],
                            bqr[:, 128 * m : 128 * (m + 1)],
                            ones1[:, :QS],
                            start=False,
                            stop=True,
                        )
                    nc.scalar.copy(qt(m), pq[:, QS * j : QS * (j + 1)])

            # ---- K projection, heads 0-7 (wk is hf-major: slice
            # wkall[:, 4H*hf + 512*kt]); heads 8-15 ride attention as filler
            for lb in range(LB):
                pk = psA.tile([128, 512], F32, tag="pv", bufs=2, name=f"pk{lb}")
                for kt in range(KT):
                    nc.tensor.matmul(
                        pk[:],
                        embt(kt)[:, 128 * lb : 128 * (lb + 1)],
                        wkall[:, 512 * kt : 512 * (kt + 1)],
                        start=(kt == 0),
                        stop=(kt == KT - 1 and not has_bias),
                    )
                if has_bias:
                    nc.tensor.matmul(
                        pk[:],
                        ones1[:, 0:128],
                        bkr[:, 0:512],
                        start=False,
                        stop=True,
                    )
                dst = kh(lb).rearrange("p (h x) -> p h x", x=65)
                nc.vector.tensor_copy(
                    dst[:, 0:8, 0:64], pk[:].rearrange("p (h x) -> p h x", x=64)
                )
                nc.gpsimd.memset(dst[:, :, 64:65], 1.0)

            # ---- V projection, pairs 0-1 (wv is m-major: pair p's weights
            # are the contiguous column group wvall[:, H*p + 128*kt])
            def vproj_big(p):
                for hf in range(2):
                    pv = psA.tile(
                        [128, 2 * QS], F32, tag="pe", bufs=2, name=f"pv{p}_{hf}"
                    )
                    for kt in range(KT):
                        for qh in range(2):
                            nc.tensor.matmul(
                                pv[:, 512 * qh : 512 * (qh + 1)],
                                wvall[:, H * p + 128 * kt : H * p + 128 * (kt + 1)],
                                embt(kt)[
                                    :,
                                    H * hf + 512 * qh : H * hf + 512 * (qh + 1),
                                ],
                                start=(kt == 0),
                                stop=(kt == KT - 1 and not has_bias),
                            )
                    if has_bias:
                        for qh in range(2):
                            nc.tensor.matmul(
                                pv[:, 512 * qh : 512 * (qh + 1)],
                                bvr[:, 128 * p : 128 * (p + 1)],
                                ones1[:, :512],
                                start=False,
                                stop=True,
                            )
                    nc.scalar.copy(vt(p)[:, H * hf : H * (hf + 1)], pv[:])

            vproj_big(0)
            vproj_big(1)

            # Q inputs die; Wo takes over wk's slot mid-attention (WAR sem)
            qkw.release()
            woall = fin.tile([128, KT * H], BF16, tag="ovl", bufs=1, name="woall")
            nc.scalar.dma_start(woall[:], wo_d[:])
            nc.scalar.dma_start(resall[:], res_d[:])
            if has_gamma:
                gam = fin.tile([128, H], F32, name="gam")
                nc.scalar.dma_start(gam[:], gam_d[:])
            if has_beta:
                bet = fin.tile([128, H], F32, name="bet")
                nc.scalar.dma_start(bet[:], bet_d[:])
            nc.gpsimd.memset(epst[:], LN_EPS)

            ap = tc.alloc_tile_pool(name="attn", bufs=1)

            pv_box = [None]

            def v_quantum(tp, c, j):
                """Two V-proj matmuls for target pair tp, seq chunk c."""
                if j == 0:
                    pv_box[0] = psA.tile(
                        [128, 512], F32, tag="pv", bufs=2, name=f"pvf{tp}_{c}"
                    )
                pv = pv_box[0]
                for kt in (2 * j, 2 * j + 1):
                    nc.tensor.matmul(
                        pv[:],
                        wvall[:, H * tp + 128 * kt : H * tp + 128 * (kt + 1)],
                        embt(kt)[:, 512 * c : 512 * (c + 1)],
                        start=(kt == 0),
                        stop=(kt == KT - 1 and not has_bias),
                    )
                if j == 3:
                    if has_bias:
                        nc.tensor.matmul(
                            pv[:],
                            bvr[:, 128 * tp : 128 * (tp + 1)],
                            ones1[:, :512],
                            start=False,
                            stop=True,
                        )
                    nc.vector.tensor_copy(
                        vt(tp)[:, 512 * c : 512 * (c + 1)], pv[:]
                    )

            def k_quantum(lb, j):
                """Two K-proj (heads 8-15) matmuls for key block lb."""
                if j == 0:
                    pv_box[0] = psA.tile(
                        [128, 512], F32, tag="pv", bufs=2, name=f"pkf{lb}"
                    )
                pk = pv_box[0]
                for kt in (2 * j, 2 * j + 1):
                    nc.tensor.matmul(
                        pk[:],
                        embt(kt)[:, 128 * lb : 128 * (lb + 1)],
                        wkall[:, 4 * H + 512 * kt : 4 * H + 512 * (kt + 1)],
                        start=(kt == 0),
                        stop=(kt == KT - 1 and not has_bias),
                    )
                if j == 3:
                    if has_bias:
                        nc.tensor.matmul(
                            pk[:],
                            ones1[:, 0:128],
                            bkr[:, 512:1024],
                            start=False,
                            stop=True,
                        )
                    nc.vector.tensor_copy(
                        kh(lb).rearrange("p (h x) -> p h x", x=65)[:, 8:16, 0:64],
                        pk[:].rearrange("p (h x) -> p h x", x=64),
                    )

            def v_chain(tp, cs):
                return [
                    (lambda tp=tp, c=c, j=j: v_quantum(tp, c, j))
                    for c in cs
                    for j in range(4)
                ]

            def k_chain(lbs):
                return [
                    (lambda lb=lb, j=j: k_quantum(lb, j))
                    for lb in lbs
                    for j in range(4)
                ]

            # pair 7 runs mid-sequence so its normalization (and every
            # out-proj t=7 matmul) resolves during attention; only pair 6's
            # norm lands in the tail. Filler work-queues keyed by POSITION;
            # each target pair's V (and kh heads 8-15 for pair 7 at pos 2)
            # is complete before that pair runs.
            pair_order = [0, 1, 7, 2, 3, 4, 5, 6]
            fill_by_pos = {
                0: k_chain(range(0, 8)) + v_chain(7, [0, 1]),
                1: k_chain(range(8, 16)) + v_chain(7, [2, 3]),
                2: v_chain(2, [0, 1, 2, 3]),
                3: v_chain(3, [0, 1, 2, 3]),
                4: v_chain(4, [0, 1, 2, 3]),
                5: v_chain(5, [0, 1, 2, 3]),
                6: v_chain(6, [0, 1, 2, 3]),
            }

            def emit_E(p, kb):
                pe2 = psA.tile(
                    [128, 2 * QS], F32, tag="pe", bufs=2, name=f"pe{p}_{kb}"
                )
                nc.tensor.matmul(
                    pe2[:, 0:QS],
                    vt(p)[0:64, 128 * kb : 128 * (kb + 1)],
                    qt(p)[0:64, :],
                    start=True,
                    stop=True,
                )
                nc.tensor.matmul(
                    pe2[:, QS : 2 * QS],
                    vt(p)[64:128, 128 * kb : 128 * (kb + 1)],
                    qt(p)[64:128, :],
                    start=True,
                    stop=True,
                )
                ex = ap.tile([128, 2 * QS], BF16, tag="ex", bufs=3, name=f"ex{p}_{kb}")
                bias = maskt[:, kb : kb + 1] if has_mask else 0.0
                nc.scalar.activation(ex[:], pe2[:], AF.Exp, bias=bias, scale=0.25)
                return ex

            def emit_att(p, kb, ex, pa):
                h0, h1 = 2 * p, 2 * p + 1
                nc.tensor.matmul(
                    pa[:, 0:QS],
                    kh(kb)[:, 65 * h0 : 65 * (h0 + 1)],
                    ex[:, 0:QS],
                    start=(kb == 0),
                    stop=(kb == LB - 1),
                )
                nc.tensor.matmul(
                    pa[:, QS : 2 * QS],
                    kh(kb)[:, 65 * h1 : 65 * (h1 + 1)],
                    ex[:, QS : 2 * QS],
                    start=(kb == 0),
                    stop=(kb == LB - 1),
                )

            def norm_stageBC(p, rcr):
                """PE broadcast of 1/sumexp + in-place normalize of apr(p).
                Deferred into the NEXT pair's kb loop so the PE never waits
                on the DVE reciprocal chain."""
                pb = psA.tile([128, 2 * QS], F32, tag="pe", bufs=2, name=f"pb{p}")
                nc.tensor.matmul(
                    pb[0:64, 0:QS],
                    ones16[64:65, 0:64],
                    rcr[64:65, 0:QS],
                    start=True,
                    stop=True,
                )
                nc.tensor.matmul(
                    pb[0:64, QS : 2 * QS],
                    ones16[64:65, 0:64],
                    rcr[64:65, QS : 2 * QS],
                    start=True,
                    stop=True,
                )
                nc.vector.tensor_mul(
                    apr(p)[0:64, :], apr(p)[0:64, :], pb[0:64, 0:QS]
                )
                nc.vector.tensor_mul(
                    apr(p)[64:128, :], apr(p)[64:128, :], pb[0:64, QS : 2 * QS]
                )

            pending_norm = [None]
            for pos in range(NP):
                p = pair_order[pos]
                pa = psA.tile([65, 2 * QS], F32, tag="pa", bufs=1, name=f"pa{p}")
                exs = [None] * LB
                work = fill_by_pos.get(pos, [])
                for kb in range(LB):
                    exs[kb] = emit_E(p, kb)
                    if kb == LB - 1 and pending_norm[0] is not None:
                        # fire the previous pair's norm here: its pe-ring
                        # slot follows pe2(kb15), so it waits only exp(kb14)
                        # and its rcr input is long ready -> no PE stall.
                        pending_norm[0]()
                        pending_norm[0] = None
                    if kb > 0:
                        emit_att(p, kb - 1, exs[kb - 1], pa)
                    for qi in range(
                        kb * len(work) // LB, (kb + 1) * len(work) // LB
                    ):
                        work[qi]()
                emit_att(p, LB - 1, exs[LB - 1], pa)
                # stage A (DVE + DMA only): evacuate unnormalized attended,
                # reciprocal of the ones-row sums, round to f32r.
                sc = ap.tile([65, 2 * QS], BF16, tag="sc", bufs=2, name=f"sc{p}")
                nc.vector.tensor_copy(sc[0:64, :], pa[0:64, :])
                rcw = ap.tile([65, 2 * QS], F32, tag="rcw", bufs=1, name=f"rcw{p}")
                nc.vector.reciprocal_approx_fast(rcw[:, :], pa[:, :])
                rcr = ap.tile([65, 2 * QS], F32R, tag="rcr", bufs=1, name=f"rcr{p}")
                nc.vector.tensor_copy(rcr[64:65, :], rcw[64:65, 0 : 2 * QS])
                nc.sync.dma_start(apr(p)[0:64, :], sc[0:64, 0:QS])
                nc.sync.dma_start(apr(p)[64:128, :], sc[0:64, QS : 2 * QS])
                pending_norm[0] = lambda p=p, rcr=rcr: norm_stageBC(p, rcr)

            if debug:
                nc.sync.dma_start(qt_dbg[:], qtall[:])
                nc.sync.dma_start(kh_dbg[:], khall[:])
                nc.sync.dma_start(vt_dbg[:], vtall[:])

            # ---- output projection + residual + LayerNorm
            wot = lambda t: woall[:, H * t : H * (t + 1)]  # noqa: E731

            t_order = [0, 1, 2, 3, 4, 5, 7, 6]
            for qcb in range(4):
                pos = [
                    psA.tile(
                        [128, 512], F32, tag="pv", bufs=2, name=f"po{qcb}_{hf}"
                    )
                    for hf in range(2)
                ]
                for ti, t in enumerate(t_order):
                    if qcb == 0 and ti == NP - 1:
                        # pair-6 (last attention pair) normalization, nested
                        # here so earlier po matmuls hoist into the ACT-bound
                        # late pairs
                        pending_norm[0]()
                        pending_norm[0] = None
                    for hf in range(2):
                        nc.tensor.matmul(
                            pos[hf][:],
                            apr(t)[:, 128 * qcb : 128 * (qcb + 1)],
                            wot(t)[:, 512 * hf : 512 * (hf + 1)],
                            start=(ti == 0),
                            stop=(ti == NP - 1),
                        )
                # LayerNorm: sums fused into the residual add; Square and the
                # final scale take the (negated) mean as an ACT bias AP.
                x = fin.tile([128, H], F32, tag="x", bufs=1, name=f"x{qcb}")
                xs = fin.tile([128, 2], F32, tag="xs", bufs=2, name=f"xs{qcb}")
                for hf in range(2):
                    nc.vector.scalar_tensor_tensor(
                        x[:, 512 * hf : 512 * (hf + 1)],
                        pos[hf][:],
                        1.0,
                        resall[:, H * qcb + 512 * hf : H * qcb + 512 * (hf + 1)],
                        op0=OP.mult,
                        op1=OP.add,
                        accum_out=xs[:, hf : hf + 1],
                    )
                mun = fin.tile([128, 1], F32, tag="mun", bufs=2, name=f"mun{qcb}")
                nc.vector.tensor_add(mun[:], xs[:, 0:1], xs[:, 1:2])
                nc.vector.tensor_scalar_mul(mun[:], mun[:], -1.0 / H)
                sq = fin.tile([128, H], F32, tag="sq", bufs=1, name=f"sq{qcb}")
                var = fin.tile([128, 1], F32, tag="var", bufs=2, name=f"var{qcb}")
                nc.scalar.activation(
                    sq[:], x[:], AF.Square, bias=mun[:], accum_out=var[:]
                )
                std = fin.tile([128, 1], F32, tag="std", bufs=2, name=f"std{qcb}")
                nc.scalar.activation(
                    std[:], var[:], AF.Sqrt, scale=1.0 / H, bias=epst[:]
                )
                rstd = fin.tile([128, 1], F32, tag="rstd", bufs=2, name=f"rstd{qcb}")
                nc.vector.reciprocal(rstd[:], std[:])
                b2 = fin.tile([128, 1], F32, tag="b2", bufs=2, name=f"b2{qcb}")
                nc.vector.tensor_mul(b2[:], mun[:], rstd[:])
                y = fin.tile([128, H], F32, tag="y", bufs=2, name=f"y{qcb}")
                nc.scalar.activation(
                    y[:], x[:], AF.Identity, bias=b2[:], scale=rstd[:]
                )
                if has_gamma:
                    nc.vector.tensor_mul(y[:], y[:], gam[:])
                if has_beta:
                    nc.vector.tensor_add(y[:], y[:], bet[:])
                oeng = nc.sync if qcb % 2 == 0 else nc.gpsimd
                oeng.dma_start(out_d[128 * qcb : 128 * (qcb + 1), :], y[:])
            if debug:
                nc.sync.dma_start(apr_dbg[:], aprall[:])
            psA.release()
            ap.release()
            fin.release()
            jp.release()

    nc.compile()
    return nc


def _bf16():
    import concourse.mybir as mybir

    return mybir.dt.np(mybir.dt.bfloat16)


def _tile_major(a, nt):
    """[nt*128, F] row-major -> [128, nt*F] tile-major (bf16)."""
    f = a.shape[1]
    return np.ascontiguousarray(
        a.reshape(nt, 128, f).transpose(1, 0, 2).reshape(128, nt * f)
    )


def stage_inputs(embeddings, mask, Wq, bq, Wk, bk, Wv, bv, Wo, bo, ln_gamma, ln_beta):
    """Build per-core in_maps (host-side layout staging)."""
    bf16 = _bf16()
    embeddings = np.asarray(embeddings, dtype=np.float32)
    mask = np.asarray(mask, dtype=np.float32)
    Wq, bq = np.asarray(Wq, np.float32), np.asarray(bq, np.float32)
    Wk, bk = np.asarray(Wk, np.float32), np.asarray(bk, np.float32)
    Wv, bv = np.asarray(Wv, np.float32), np.asarray(bv, np.float32)
    Wo, bo = np.asarray(Wo, np.float32), np.asarray(bo, np.float32)
    ln_gamma = np.asarray(ln_gamma, np.float32)
    ln_beta = np.asarray(ln_beta, np.float32)

    has_bias = bool(np.any(bq) or np.any(bk) or np.any(bv))
    has_mask = bool(np.any(mask))
    has_gamma = bool(np.any(ln_gamma != 1.0))
    has_beta = bool(np.any(ln_beta))
    key = (has_bias, has_mask, has_gamma, has_beta)

    # wq staged m-major: [p, m*H + kt*128 + c] = Wq[kt*128+p, m*128+c]
    wq_s = np.ascontiguousarray(
        Wq.astype(bf16)
        .reshape(KT, 128, KT, 128)
        .transpose(1, 2, 0, 3)
        .reshape(128, KT * H)
    )
    # wk staged hf-major: [p, hf*4H + kt*512 + c] = Wk[kt*128+p, hf*512+c]
    wk_s = np.ascontiguousarray(
        Wk.astype(bf16)
        .reshape(KT, 128, 2, 512)
        .transpose(1, 2, 0, 3)
        .reshape(128, KT * H)
    )
    # wv staged m-major like wq
    wv_s = np.ascontiguousarray(
        Wv.astype(bf16)
        .reshape(KT, 128, KT, 128)
        .transpose(1, 2, 0, 3)
        .reshape(128, KT * H)
    )
    wo_s = _tile_major(Wo.astype(bf16), KT)
    ones_s = np.ones((128, 64), dtype=np.float32)

    in_maps = []
    for c in range(NC):
        b, s = c // 4, c % 4
        e = embeddings[b]  # (L, H)
        embT = np.ascontiguousarray(e.T).astype(bf16)  # (H, L) bf16
        embq_s = _tile_major(
            np.ascontiguousarray(embT[:, QS * s : QS * (s + 1)]), KT
        )
        # embt: quarter-major [128, (q t l)] so each quarter is 1 contiguous DMA
        embt_s = np.ascontiguousarray(
            embT.reshape(KT, 128, 4, 512)
            .transpose(1, 2, 0, 3)
            .reshape(128, 4 * KT * 512)
        )
        res = (e[QS * s : QS * (s + 1)] + bo[None, :]).astype(np.float32)
        res_s = np.ascontiguousarray(
            res.reshape(4, 128, H).transpose(1, 0, 2).reshape(128, 4 * H)
        )
        m = {
            "embqS": embq_s,
            "embtS": embt_s,
            "wqS": wq_s,
            "wkS": wk_s,
            "wvS": wv_s,
            "woS": wo_s,
            "onescol": ones_s,
            "resS": res_s,
        }
        if has_mask:
            m["maskpk"] = np.ascontiguousarray(mask[b, 0, 0].reshape(LB, 128).T)
        if has_bias:
            m["bqr"] = bq[None, :].astype(bf16)
            m["bkr"] = bk[None, :].astype(bf16)
            m["bvr"] = bv[None, :].astype(bf16)
            m["onesrow"] = np.ones((1, H), dtype=bf16)
        if has_gamma:
            m["gam"] = np.broadcast_to(ln_gamma, (128, H)).astype(np.float32).copy()
        if has_beta:
            m["bet"] = np.broadcast_to(ln_beta, (128, H)).astype(np.float32).copy()
        in_maps.append(m)
    return key, in_maps


def kernel(embeddings, mask, Wq, bq, Wk, bk, Wv, bv, Wo, bo, ln_gamma, ln_beta):
    from concourse.bass_utils import run_bass_kernel_spmd

    key, in_maps = stage_inputs(
        embeddings, mask, Wq, bq, Wk, bk, Wv, bv, Wo, bo, ln_gamma, ln_beta
    )
    if key not in _programs:
        _programs[key] = _build(*key)
    nc = _programs[key]

    r = run_bass_kernel_spmd(nc, in_maps, list(range(NC)))
    out = np.empty((B, L, H), dtype=np.float32)
    for c in range(NC):
        b, s = c // 4, c % 4
        out[b, QS * s : QS * (s + 1)] = r.results[c]["out"]
    return out
